# revision 1
# baseline (speedup 1.0000x reference)
"""Causal self-attention (QKV projection + softmax(QK^T/sqrt(N)) @ V) on 8 TRN2
NeuronCores.

Sharding: core c = 2*b + j handles batch element b (of 4) and half the query
rows. For causal load balance, each core takes two 512-row query blocks from
both ends of the triangle: block A = rows [j*512,(j+1)*512), block B = rows
[(3-j)*512,(4-j)*512).  Uniform SPMD schedule: block A attends k-tiles 0..7,
block B attends k-tiles 0..15; per-core causal masks (built on-device from
shipped position vectors) zero out the invalid/extra tiles.

Layout trick: context is shipped pre-transposed [D, N], so Q^T, K^T come out
of the projection directly in [e, n] layout and V in [n, e] layout; scores are
computed transposed S^T[k, q] = K^T.T @ Q^T, softmax runs without max-
subtraction (scores/sqrt(2048) are tiny), the per-query denominator comes from
a ones-vector matmul, and P^T is exactly the lhsT that PV needs. Zero on-chip
transposes. All matmuls in float32r (FP22 truncated, full-rate).
"""

import math
from contextlib import ExitStack

import numpy as np

import concourse.bass as bass
import concourse.mybir as mybir
import concourse.tile as tile
from concourse.bass_utils import run_bass_kernel_spmd
from concourse.tile_rust import add_dep_helper

P = 128
CH = 512  # free-dim chunk (max fp32 moving operand / one PSUM bank)


def _chunks(total, size):
    return [(o, min(size, total - o)) for o in range(0, total, size)]


def _fix_matmul_waits(nc):
    """Walrus codegen has a small per-instruction sync-wait slot budget (one
    for a self-loading float32r matmul's LDWEIGHTS half, similar for ACT etc).
    Move extra waits onto NoOps inserted just before the instruction on the
    same engine — per-engine program order (and thus semantics) is unchanged."""
    import concourse.mybir as mybir
    skip = (mybir.InstEventSemaphore, mybir.InstNoOp,
            mybir.InstUnconditionalBranch, mybir.InstCall)
    for func in nc.m.functions:
        for bb in func.blocks:
            il = bb.instructions
            new = []
            changed = False
            for inst in il:
                si = getattr(inst, "sync_info", None)
                if (si and si.on_wait and len(si.on_wait) > 1
                        and not isinstance(inst, skip)):
                    waits = list(si.on_wait)
                    for wi, w in enumerate(waits[:-1]):
                        nop = mybir.InstNoOp(
                            name=f"{inst.name}-wfix{wi}", engine=inst.engine,
                            sync_info=mybir.SyncInfo(on_wait=[w], on_update=[]),
                            text_hint="waitfix")
                        new.append(nop)
                    inst.sync_info = mybir.SyncInfo(
                        on_wait=[waits[-1]], on_update=list(si.on_update or []))
                    changed = True
                new.append(inst)
            if changed:
                bb.instructions = new


def build(N=2048, D=1024, n_cores=8, fix_waits=True, **bass_kwargs):
    NT = N // P          # number of 128-row key tiles
    DN = D // P          # contraction tiles (and e-tiles of Q/K)
    QBLK = N // 4        # rows per query block
    QT = QBLK // P       # q-tiles per query block
    QTOT = 2 * QBLK      # query rows per core
    SCALE = 1.0 / math.sqrt(N)
    FR = mybir.dt.float32r
    F32 = mybir.dt.float32
    AF = mybir.ActivationFunctionType
    OP = mybir.AluOpType

    nc = bass.Bass(**bass_kwargs)
    anchors = []  # first K-proj matmul of each ctx chunk; DMA stage gates
    kends = []   # last K-proj matmul of each ctx chunk

    def _after(dma_bi, anchor_idx, lst=None):
        """Gate a bulk DMA behind an earlier compute anchor so concurrent
        transfers don't fair-share-starve the startup-critical ones."""
        lst = anchors if lst is None else lst
        if lst and anchor_idx < len(lst):
            add_dep_helper(dma_bi.ins, lst[anchor_idx].ins, sync=True,
                           reason="dma staging")
        return dma_bi

    ctx_kvT = nc.declare_dram_parameter("ctx_kvT", [D, N], FR, isOutput=False)
    ctx_qT = nc.declare_dram_parameter("ctx_qT", [D, QTOT], FR, isOutput=False)
    w_qkv = nc.declare_dram_parameter("w_qkv", [D, 3 * D], FR, isOutput=False)
    qpos = nc.declare_dram_parameter("qpos", [P, QTOT], F32, isOutput=False)
    kpos = nc.declare_dram_parameter("kpos", [P, NT], F32, isOutput=False)
    bqT = nc.declare_dram_parameter("bqT", [P, DN], FR, isOutput=False)
    bkT = nc.declare_dram_parameter("bkT", [P, DN], FR, isOutput=False)
    bvb = nc.declare_dram_parameter("bvb", [P, D], FR, isOutput=False)
    onesd = nc.declare_dram_parameter("onesd", [P, 8], FR, isOutput=False)
    out_ext = nc.declare_dram_parameter("out", [QTOT, D], FR, isOutput=True)

    with ExitStack() as ctx:
        tc = ctx.enter_context(tile.TileContext(nc))
        const = ctx.enter_context(tc.tile_pool(name="const", bufs=1))
        persist = ctx.enter_context(tc.tile_pool(name="persist", bufs=1))
        dram = ctx.enter_context(tc.tile_pool(name="dram", bufs=1, space="DRAM"))

        qpos_sb = const.tile([P, QTOT], F32)
        kpos_sb = const.tile([P, NT], F32)
        bq_sb = const.tile([P, DN], FR)
        nc.sync.dma_start(out=bq_sb, in_=bqT[:, :])
        bk_sb = const.tile([P, DN], FR)
        nc.sync.dma_start(out=bk_sb, in_=bkT[:, :])
        bv_sb = const.tile([P, D], FR)
        nc.sync.dma_start(out=bv_sb, in_=bvb[:, :])
        ones_sb = const.tile([P, 8], FR)
        nc.sync.dma_start(out=ones_sb, in_=onesd[:, :])

        # ---------------- K/V projection (ctx_kvT read once) ----------------
        # K^T staged to DRAM as [NT, D, P] tiles (streamed back during scores);
        # V kept resident in SBUF (PV re-reads it twice and is latency-critical).
        k_dram = dram.tile([NT, D, P], FR, name="k_dram")
        v_sb = [persist.tile([P, D], FR, tag=f"v{t}", name=f"v{t}") for t in range(NT)]

        with tc.tile_pool(name="wkv", bufs=1) as wkv, \
             tc.tile_pool(name="ctxp", bufs=3) as ctxp, \
             tc.tile_pool(name="evict", bufs=3) as evict, \
             tc.tile_pool(name="pp", bufs=8, space="PSUM") as pp:
            wk_sb = [wkv.tile([P, D], FR, tag=f"wk{d}", name=f"wk{d}") for d in range(DN)]
            wv_sb = [wkv.tile([P, D], FR, tag=f"wv{d}", name=f"wv{d}") for d in range(DN)]
            # startup-critical DMA order: W_k, then first ctx chunk, then W_v
            HF = D // 2
            for d in range(DN):
                nc.sync.dma_start(out=wk_sb[d][:, 0:HF], in_=w_qkv[d * P:(d + 1) * P, D:D + HF])
            first_cts = []
            for d in range(DN):
                ct = ctxp.tile([P, CH], FR, tag=f"ct{d}", name=f"ct{d}")
                stage0_last = nc.sync.dma_start(out=ct, in_=ctx_kvT[d * P:(d + 1) * P, 0:CH])
                first_cts.append(ct)
            # second wk halves and wv: chained behind the startup burst so the
            # critical 4MB (wk first halves + first ctx chunk) gets full HBM rate
            for d in range(DN):
                wk2 = nc.sync.dma_start(out=wk_sb[d][:, HF:D], in_=w_qkv[d * P:(d + 1) * P, D + HF:2 * D])
                add_dep_helper(wk2.ins, stage0_last.ins, sync=True, reason="dma staging")
            for d in range(DN):
                wvd = nc.sync.dma_start(out=wv_sb[d], in_=w_qkv[d * P:(d + 1) * P, 2 * D:3 * D])
                add_dep_helper(wvd.ins, wk2.ins, sync=True, reason="dma staging")
            for ci, (coff, csz) in enumerate(_chunks(N, CH)):
                cbase = 0
                if ci == 0:
                    cts = first_cts
                else:
                    cts = []
                    for d in range(DN):
                        ct = ctxp.tile([P, CH], FR, tag=f"ct{d}", name=f"ct{d}")
                        _after(nc.sync.dma_start(out=ct[:, :csz],
                                                 in_=ctx_kvT[d * P:(d + 1) * P, coff:coff + csz]), ci - 1,
                               lst=kends)
                        cts.append(ct)
                for e in range(DN):
                    psk = pp.tile([P, CH], F32, tag="pp8", name="psk")
                    for d in range(DN):
                        mm = nc.tensor.matmul(psk[:, :csz], lhsT=wk_sb[d][:, e * P:(e + 1) * P],
                                              rhs=cts[d][:, cbase:cbase + csz], start=(d == 0), stop=(d == DN - 1))
                        if e == 0 and d == 0:
                            anchors.append(mm)
                        if e == DN - 1 and d == DN - 1:
                            kends.append(mm)
                    kev = evict.tile([P, CH], FR, tag="kev", name="kev")
                    nc.scalar.activation(kev[:, :csz], psk[:, :csz],
                                         AF.Identity, bias=bk_sb[:, e:e + 1], scale=1.0)
                    for i in range(csz // P):
                        kt = coff // P + i
                        nc.scalar.dma_start(out=k_dram[kt, e * P:(e + 1) * P, :],
                                            in_=kev[:, i * P:(i + 1) * P])
                for nt_loc in range(csz // P):
                    n_t = coff // P + nt_loc
                    for eoff, esz in _chunks(D, CH):
                        psv = pp.tile([P, CH], F32, tag="pp8", name="psv")
                        for d in range(DN):
                            nc.tensor.matmul(psv[:, :esz],
                                             lhsT=cts[d][:, cbase + nt_loc * P:cbase + (nt_loc + 1) * P],
                                             rhs=wv_sb[d][:, eoff:eoff + esz], start=(d == 0), stop=(d == DN - 1))
                        nc.vector.tensor_tensor(v_sb[n_t][:, eoff:eoff + esz], psv[:, :esz],
                                                bv_sb[:, eoff:eoff + esz], OP.add)

        # ---------------- attention (with per-block Q projection) ----------------
        with tc.tile_pool(name="wq", bufs=1) as wqp, \
             tc.tile_pool(name="ctxq", bufs=1) as ctxq, \
             tc.tile_pool(name="qtb", bufs=1) as qtb, \
             tc.tile_pool(name="kstream", bufs=4) as kpool, \
             tc.tile_pool(name="att_e", bufs=1) as epool, \
             tc.tile_pool(name="att_m", bufs=3) as mpool, \
             tc.tile_pool(name="att_o", bufs=3) as opool, \
             tc.tile_pool(name="ps_s", bufs=2, space="PSUM") as ps_s, \
             tc.tile_pool(name="ps_pv", bufs=4, space="PSUM") as ps_pv, \
             tc.tile_pool(name="ps_den", bufs=2, space="PSUM") as ps_den:
            wq_sb = [wqp.tile([P, D], FR, tag=f"wq{d}", name=f"wq{d}") for d in range(DN)]
            for d in range(DN):
                _after(nc.sync.dma_start(out=wq_sb[d], in_=w_qkv[d * P:(d + 1) * P, 0:D]),
                       max(0, len(anchors) - 2))
            _after(nc.sync.dma_start(out=qpos_sb, in_=qpos[:, :]), 1)
            _after(nc.sync.dma_start(out=kpos_sb, in_=kpos[:, :]), 1)
            e_sb = [epool.tile([P, QBLK], FR, tag=f"e{k}", name=f"e{k}") for k in range(NT)]
            qT_sb = [qtb.tile([P, QBLK], FR, tag=f"qtb{e}", name=f"qtb{e}") for e in range(DN)]
            for qb in range(2):
                KT = NT // 2 if qb == 0 else NT
                qoff = qb * QBLK
                # Q projection for this block only
                cqs = []
                for d in range(DN):
                    cq = ctxq.tile([P, QBLK], FR, tag=f"cq{d}", name=f"cq{d}")
                    _after(nc.sync.dma_start(out=cq, in_=ctx_qT[d * P:(d + 1) * P, qoff:qoff + QBLK]),
                           max(0, len(anchors) - 2 + qb))
                    cqs.append(cq)
                for e in range(DN):
                    psq = ps_s.tile([P, QBLK], F32, tag="s", name="psq")
                    for d in range(DN):
                        nc.tensor.matmul(psq, lhsT=wq_sb[d][:, e * P:(e + 1) * P],
                                         rhs=cqs[d], start=(d == 0), stop=(d == DN - 1))
                    nc.scalar.activation(qT_sb[e], psq, AF.Identity,
                                         bias=bq_sb[:, e:e + 1], scale=1.0)
                # scores + exp + mask (K^T streamed back from DRAM)
                for k in range(KT):
                    ksb = kpool.tile([P, D], FR, tag="ksb", name="ksb")
                    nc.sync.dma_start(
                        out=ksb.rearrange("p (dt c) -> p dt c", c=P),
                        in_=k_dram[k].rearrange("(dt p) c -> p dt c", p=P))
                    pss = ps_s.tile([P, QBLK], F32, tag="s", name="pss")
                    for d in range(DN):
                        nc.tensor.matmul(pss, lhsT=ksb[:, d * P:(d + 1) * P],
                                         rhs=qT_sb[d], start=(d == 0), stop=(d == DN - 1))
                    nc.scalar.activation(e_sb[k], pss, AF.Exp, scale=SCALE)
                    if qb == 0 or k >= NT // 2:
                        m = mpool.tile([P, QBLK], F32, tag="m", name="m")
                        nc.vector.tensor_scalar(m, qpos_sb[:, qoff:qoff + QBLK],
                                                kpos_sb[:, k:k + 1], None, OP.is_ge)
                        nc.vector.tensor_tensor(e_sb[k], e_sb[k], m, OP.mult)
                # PV in q-tile groups of 2 (V is SBUF-resident: no DMA here)
                for qt in range(QT):
                    pso = [ps_pv.tile([P, CH], F32, tag="pv", name="pso") for _ in _chunks(D, CH)]
                    psd = ps_den.tile([P, 8], F32, tag="den", name="psd")
                    for k in range(KT):
                        lhsT = e_sb[k][:, qt * P:(qt + 1) * P]
                        for ei, (eoff, esz) in enumerate(_chunks(D, CH)):
                            nc.tensor.matmul(pso[ei][:, :esz], lhsT=lhsT,
                                             rhs=v_sb[k][:, eoff:eoff + esz],
                                             start=(k == 0), stop=(k == KT - 1))
                        nc.tensor.matmul(psd, lhsT=lhsT, rhs=ones_sb,
                                         start=(k == 0), stop=(k == KT - 1))
                    rec = mpool.tile([P, 1], F32, tag="rec", name="rec")
                    nc.vector.reciprocal(rec, psd[:, 0:1])
                    for ei, (eoff, esz) in enumerate(_chunks(D, CH)):
                        ot = opool.tile([P, CH], FR, tag="o", name="ot")
                        nc.vector.tensor_scalar_mul(ot[:, :esz], pso[ei][:, :esz], rec)
                        nc.scalar.dma_start(out=out_ext[qoff + qt * P:qoff + (qt + 1) * P, eoff:eoff + esz],
                                            in_=ot[:, :esz])
    if fix_waits:
        _fix_matmul_waits(nc)
    return nc


def make_in_maps(context, W_qkv, b_qkv, n_cores=8):
    context = np.ascontiguousarray(np.asarray(context, np.float32))
    W_qkv = np.ascontiguousarray(np.asarray(W_qkv, np.float32))
    b_qkv = np.ascontiguousarray(np.asarray(b_qkv, np.float32))
    B, N, D = context.shape
    NT = N // P
    DN = D // P
    QBLK = N // 4
    QTOT = 2 * QBLK
    kpos = (np.arange(NT)[None, :] * P + np.arange(P)[:, None]).astype(np.float32)
    kpos = np.ascontiguousarray(kpos)
    bq = np.ascontiguousarray(b_qkv[0:D].reshape(DN, P).T)
    bk = np.ascontiguousarray(b_qkv[D:2 * D].reshape(DN, P).T)
    bv = np.ascontiguousarray(np.broadcast_to(b_qkv[2 * D:3 * D], (P, D)))
    in_maps = []
    for c in range(n_cores):
        b, j = divmod(c, 2)
        sA = slice(j * QBLK, (j + 1) * QBLK)
        sB = slice((3 - j) * QBLK, (4 - j) * QBLK)
        ctx_b = context[b]
        ctx_kvT = np.ascontiguousarray(ctx_b.T)
        ctx_qT = np.ascontiguousarray(np.concatenate([ctx_b[sA], ctx_b[sB]], axis=0).T)
        qpos_row = np.concatenate([np.arange(sA.start, sA.stop), np.arange(sB.start, sB.stop)])
        qpos_b = np.ascontiguousarray(np.broadcast_to(qpos_row.astype(np.float32), (P, QTOT)))
        in_maps.append({
            "ctx_kvT": ctx_kvT, "ctx_qT": ctx_qT, "w_qkv": W_qkv,
            "qpos": qpos_b, "kpos": kpos, "bqT": bq, "bkT": bk, "bvb": bv,
            "onesd": np.ones((P, 8), np.float32),
        })
    return in_maps


def assemble(results, B, N, D):
    QBLK = N // 4
    out = np.zeros((B, N, D), np.float32)
    for c, res in enumerate(results):
        b, j = divmod(c, 2)
        o = np.asarray(res["out"], np.float32)
        out[b, j * QBLK:(j + 1) * QBLK] = o[:QBLK]
        out[b, (3 - j) * QBLK:(4 - j) * QBLK] = o[QBLK:]
    return out


def run(inputs, trace=False, **spmd_kwargs):
    context = np.asarray(inputs["context"])
    B, N, D = context.shape
    nc = build(N, D)
    in_maps = make_in_maps(context, inputs["W_qkv"], inputs["b_qkv"], n_cores=8)
    res = run_bass_kernel_spmd(nc, in_maps, core_ids=list(range(8)), trace=trace, **spmd_kwargs)
    out = assemble(res.results, B, N, D)
    return out, res


def kernel(context, W_qkv, b_qkv):
    out, _ = run({"context": context, "W_qkv": W_qkv, "b_qkv": b_qkv})
    return out



# revision 5
# speedup vs baseline: 1.0473x; 1.0473x over previous
"""Causal self-attention (QKV projection + softmax(QK^T/sqrt(N)) @ V) on 8 TRN2
NeuronCores.

Sharding: core c = 2*b + j handles batch element b (of 4) and half the query
rows (two 512-row blocks from opposite ends of the causal triangle). All
operands are bf16 (f32 PSUM accumulation): numpy-simulated end-to-end rel err
is ~4e-3 vs the 2e-2 gate, and it halves every DMA stream.

Uniform-SPMD trick: each core processes the four 512-token context chunks in a
per-core PERMUTED order (j=0: [0,1,2,3]; j=1: [1,0,3,2]) so that the chunk at
schedule position 0 is always the core's low query block and position 3 its
high query block. The Q projection runs on the in-SBUF ctx chunk tiles at
schedule positions 1 (on the saved pos-0 chunk, keeping W_q off the startup
critical path) and 3 — no separate ctx_qT input and no KV->attention DMA stall.
K^T, V, Q^T and the exp'd scores all stay SBUF-resident in bf16 (~12MB), so
the attention phase runs with zero HBM traffic except output writes. Scores
are computed transposed (S^T = K^T.T @ Q^T) so softmax needs no transposes;
denominators come from a ones-vector matmul; per-core causal masks built
on-device from shipped position vectors make the uniform schedule correct.
"""

import math
from contextlib import ExitStack

import numpy as np

import concourse.bass as bass
import concourse.mybir as mybir
import concourse.tile as tile
from concourse.bass_utils import run_bass_kernel_spmd

P = 128
CH = 512  # ctx chunk columns == query block rows == max f32 PSUM free dim


def _fix_matmul_waits(nc):
    """Walrus codegen has a small per-instruction sync-wait slot budget (one
    for a self-loading matmul's LDWEIGHTS half, similar for ACT etc). Move
    extra waits onto NoOps inserted just before the instruction on the same
    engine — per-engine program order (and thus semantics) is unchanged."""
    skip = (mybir.InstEventSemaphore, mybir.InstNoOp,
            mybir.InstUnconditionalBranch, mybir.InstCall)
    for func in nc.m.functions:
        for bb in func.blocks:
            il = bb.instructions
            new = []
            changed = False
            for inst in il:
                si = getattr(inst, "sync_info", None)
                if (si and si.on_wait and len(si.on_wait) > 1
                        and not isinstance(inst, skip)):
                    waits = list(si.on_wait)
                    for wi, w in enumerate(waits[:-1]):
                        nop = mybir.InstNoOp(
                            name=f"{inst.name}-wfix{wi}", engine=inst.engine,
                            sync_info=mybir.SyncInfo(on_wait=[w], on_update=[]),
                            text_hint="waitfix")
                        new.append(nop)
                    inst.sync_info = mybir.SyncInfo(
                        on_wait=[waits[-1]], on_update=list(si.on_update or []))
                    changed = True
                new.append(inst)
            if changed:
                bb.instructions = new


def build(N=2048, D=1024, fix_waits=True, **bass_kwargs):
    NT = N // P          # schedule k-tiles
    DN = D // P          # contraction blocks (and e-blocks of Q/K)
    NCH = N // CH        # ctx chunks (== 4 == query blocks per batch)
    QBLK = CH            # rows per query block
    QT = QBLK // P
    QTOT = 2 * QBLK
    SCALE = 1.0 / math.sqrt(N)
    BF = mybir.dt.bfloat16
    F32 = mybir.dt.float32
    AF = mybir.ActivationFunctionType
    OP = mybir.AluOpType

    nc = bass.Bass(**bass_kwargs)

    ctxT = nc.declare_dram_parameter("ctxT", [D, N], BF, isOutput=False)
    w_qkv = nc.declare_dram_parameter("w_qkv", [D, 3 * D], BF, isOutput=False)
    qpos = nc.declare_dram_parameter("qpos", [P, QTOT], F32, isOutput=False)
    kpos = nc.declare_dram_parameter("kpos", [P, NT], F32, isOutput=False)
    bqT = nc.declare_dram_parameter("bqT", [P, DN], F32, isOutput=False)
    bkT = nc.declare_dram_parameter("bkT", [P, DN], F32, isOutput=False)
    bvb = nc.declare_dram_parameter("bvb", [P, D], F32, isOutput=False)
    onesd = nc.declare_dram_parameter("onesd", [P, 8], BF, isOutput=False)
    out_ext = nc.declare_dram_parameter("out", [QTOT, D], BF, isOutput=True)

    with ExitStack() as ctx:
        tc = ctx.enter_context(tile.TileContext(nc))
        const = ctx.enter_context(tc.tile_pool(name="const", bufs=1))
        persist = ctx.enter_context(tc.tile_pool(name="persist", bufs=1))

        # persistent stores (all bf16):
        #   k_store col (d*N + n): K^T e-block d, key n           (4MB)
        #   v_store col (kt*D + e): V of schedule k-tile kt        (4MB)
        #   q_store col (e*QTOT + qb*QBLK + q): Q^T e-block e      (2MB)
        k_store = persist.tile([P, DN * N], BF, name="k_store")
        v_store = persist.tile([P, NT * D], BF, name="v_store")
        q_store = persist.tile([P, DN * QTOT], BF, name="q_store")

        bk_sb = const.tile([P, DN], F32)
        bq_sb = const.tile([P, DN], F32)
        bv_sb = const.tile([P, D], F32)
        ones_sb = const.tile([P, 8], BF)
        qpos_sb = const.tile([P, QTOT], F32)
        kpos_sb = const.tile([P, NT], F32)

        with tc.tile_pool(name="wts", bufs=1) as wts, \
             tc.tile_pool(name="ctxp", bufs=3) as ctxp, \
             tc.tile_pool(name="pp", bufs=8, space="PSUM") as pp:
            # weight stores: wk/wq e-major (lhsT tile (e,d) at col (e*DN+d)*P),
            # wv d-major (rhs slice (d, eoff) at col d*D+eoff)
            wk_sb = wts.tile([P, D * DN], BF, name="wk")
            wq_sb = wts.tile([P, D * DN], BF, name="wq")
            wv_sb = wts.tile([P, D * DN], BF, name="wv")
            cts = [ctxp.tile([P, DN * CH], BF, tag="ct", name=f"ct{c}")
                   for c in range(NCH)]

            # ---- DMA stream, in exact consumption order (one sync ring) ----
            def ld_w(dst, e, col0):  # one e-column-block of a weight matrix
                nc.sync.dma_start(
                    out=dst[:, e * D:(e + 1) * D].rearrange("p (d c) -> p d c", c=P),
                    in_=w_qkv[:, col0 + e * P:col0 + (e + 1) * P]
                        .rearrange("(d p) c -> p d c", p=P))

            def ld_ctx(pos):
                nc.sync.dma_start(
                    out=cts[pos].rearrange("p (d c) -> p d c", c=CH),
                    in_=ctxT[:, pos * CH:(pos + 1) * CH]
                        .rearrange("(d p) c -> p d c", p=P))

            nc.sync.dma_start(out=bk_sb, in_=bkT[:, :])
            ld_w(wk_sb, 0, D)            # first K-proj needs only e-block 0
            ld_ctx(0)
            for e in range(1, DN):
                ld_w(wk_sb, e, D)
            for d in range(DN):          # wv d-major: col d*D + e, plain 2D
                nc.sync.dma_start(out=wv_sb[:, d * D:(d + 1) * D],
                                  in_=w_qkv[d * P:(d + 1) * P, 2 * D:3 * D])
            nc.sync.dma_start(out=bv_sb, in_=bvb[:, :])
            nc.sync.dma_start(out=ones_sb, in_=onesd[:, :])
            nc.sync.dma_start(out=bq_sb, in_=bqT[:, :])
            ld_ctx(1)
            for e in range(DN):
                ld_w(wq_sb, e, 0)
            nc.sync.dma_start(out=qpos_sb, in_=qpos[:, :])
            nc.sync.dma_start(out=kpos_sb, in_=kpos[:, :])
            ld_ctx(2)
            ld_ctx(3)

            def kv_proj(pos):
                for e in range(DN):
                    psk = pp.tile([P, CH], F32, tag="pp8", name="psk")
                    for d in range(DN):
                        nc.tensor.matmul(
                            psk, lhsT=wk_sb[:, (e * DN + d) * P:(e * DN + d + 1) * P],
                            rhs=cts[pos][:, d * CH:(d + 1) * CH],
                            start=(d == 0), stop=(d == DN - 1))
                    nc.scalar.activation(
                        k_store[:, e * N + pos * CH:e * N + (pos + 1) * CH],
                        psk, AF.Identity, bias=bk_sb[:, e:e + 1], scale=1.0)
                for nt_loc in range(CH // P):
                    kt = pos * (CH // P) + nt_loc
                    for ei in range(D // CH):
                        psv = pp.tile([P, CH], F32, tag="pp8", name="psv")
                        for d in range(DN):
                            nc.tensor.matmul(
                                psv,
                                lhsT=cts[pos][:, d * CH + nt_loc * P:d * CH + (nt_loc + 1) * P],
                                rhs=wv_sb[:, d * D + ei * CH:d * D + (ei + 1) * CH],
                                start=(d == 0), stop=(d == DN - 1))
                        nc.vector.tensor_tensor(
                            v_store[:, kt * D + ei * CH:kt * D + (ei + 1) * CH],
                            psv, bv_sb[:, ei * CH:(ei + 1) * CH], OP.add)

            def q_proj(pos, qb):
                for e in range(DN):
                    psq = pp.tile([P, CH], F32, tag="pp8", name="psq")
                    for d in range(DN):
                        nc.tensor.matmul(
                            psq, lhsT=wq_sb[:, (e * DN + d) * P:(e * DN + d + 1) * P],
                            rhs=cts[pos][:, d * CH:(d + 1) * CH],
                            start=(d == 0), stop=(d == DN - 1))
                    nc.scalar.activation(
                        q_store[:, e * QTOT + qb * QBLK:e * QTOT + (qb + 1) * QBLK],
                        psq, AF.Identity, bias=bq_sb[:, e:e + 1], scale=1.0)

            kv_proj(0)
            kv_proj(1)
            q_proj(0, 0)
            kv_proj(2)
            kv_proj(3)
            q_proj(3, 1)

        # ---------------- attention (all operands SBUF-resident) ----------------
        with tc.tile_pool(name="att_e", bufs=1) as epool, \
             tc.tile_pool(name="att_m", bufs=3) as mpool, \
             tc.tile_pool(name="att_o", bufs=3) as opool, \
             tc.tile_pool(name="ps_s", bufs=2, space="PSUM") as ps_s, \
             tc.tile_pool(name="ps_pv", bufs=4, space="PSUM") as ps_pv, \
             tc.tile_pool(name="ps_den", bufs=2, space="PSUM") as ps_den:
            e_all = epool.tile([P, NT * QBLK], BF, name="e_all")
            for qb in range(2):
                KT = NT // 2 if qb == 0 else NT
                for k in range(KT):
                    pss = ps_s.tile([P, QBLK], F32, tag="s", name="pss")
                    for d in range(DN):
                        nc.tensor.matmul(
                            pss, lhsT=k_store[:, d * N + k * P:d * N + (k + 1) * P],
                            rhs=q_store[:, d * QTOT + qb * QBLK:d * QTOT + (qb + 1) * QBLK],
                            start=(d == 0), stop=(d == DN - 1))
                    esl = e_all[:, k * QBLK:(k + 1) * QBLK]
                    nc.scalar.activation(esl, pss, AF.Exp, scale=SCALE)
                    if qb == 0 or k >= NT // 2:
                        m = mpool.tile([P, QBLK], BF, tag="m", name="m")
                        nc.vector.tensor_scalar(m, qpos_sb[:, qb * QBLK:(qb + 1) * QBLK],
                                                kpos_sb[:, k:k + 1], None, OP.is_ge)
                        nc.vector.tensor_tensor(esl, esl, m, OP.mult)
                for qt in range(QT):
                    pso = [ps_pv.tile([P, CH], F32, tag="pv", name="pso")
                           for _ in range(D // CH)]
                    psd = ps_den.tile([P, 8], F32, tag="den", name="psd")
                    for k in range(KT):
                        lhsT = e_all[:, k * QBLK + qt * P:k * QBLK + (qt + 1) * P]
                        for ei in range(D // CH):
                            nc.tensor.matmul(pso[ei], lhsT=lhsT,
                                             rhs=v_store[:, k * D + ei * CH:k * D + (ei + 1) * CH],
                                             start=(k == 0), stop=(k == KT - 1))
                        nc.tensor.matmul(psd, lhsT=lhsT, rhs=ones_sb,
                                         start=(k == 0), stop=(k == KT - 1))
                    rec = mpool.tile([P, 1], F32, tag="rec", name="rec")
                    nc.vector.reciprocal(rec, psd[:, 0:1])
                    for ei in range(D // CH):
                        ot = opool.tile([P, CH], BF, tag="o", name="ot")
                        nc.vector.tensor_scalar_mul(ot, pso[ei], rec)
                        nc.scalar.dma_start(
                            out=out_ext[qb * QBLK + qt * P:qb * QBLK + (qt + 1) * P,
                                        ei * CH:(ei + 1) * CH],
                            in_=ot)
    if fix_waits:
        _fix_matmul_waits(nc)
    return nc


def _chunk_order(j, nch):
    # position 0 = low query block, position 3 = high query block, per core.
    return [0, 1, 2, 3] if j == 0 else [1, 0, 3, 2]


def make_in_maps(context, W_qkv, b_qkv, n_cores=8):
    import ml_dtypes
    bf16 = ml_dtypes.bfloat16
    context = np.ascontiguousarray(np.asarray(context, np.float32))
    W_qkv = np.asarray(W_qkv, np.float32).astype(bf16)
    b_qkv = np.ascontiguousarray(np.asarray(b_qkv, np.float32))
    B, N, D = context.shape
    NT = N // P
    DN = D // P
    NCH = N // CH
    QTOT = 2 * CH
    bq = np.ascontiguousarray(b_qkv[0:D].reshape(DN, P).T)
    bk = np.ascontiguousarray(b_qkv[D:2 * D].reshape(DN, P).T)
    bv = np.ascontiguousarray(np.broadcast_to(b_qkv[2 * D:3 * D], (P, D)))
    in_maps = []
    for c in range(n_cores):
        b, j = divmod(c, 2)
        order = _chunk_order(j, NCH)
        ctx_bT = context[b].T.astype(bf16)  # [D, N]
        ctxT = np.ascontiguousarray(np.concatenate(
            [ctx_bT[:, o * CH:(o + 1) * CH] for o in order], axis=1))
        qpos_row = np.concatenate([
            np.arange(order[0] * CH, (order[0] + 1) * CH),
            np.arange(order[3] * CH, (order[3] + 1) * CH)]).astype(np.float32)
        qpos_b = np.ascontiguousarray(np.broadcast_to(qpos_row, (P, QTOT)))
        kpos = np.empty((P, NT), np.float32)
        for t in range(NT):
            kpos[:, t] = order[t // 4] * CH + (t % 4) * P + np.arange(P)
        in_maps.append({
            "ctxT": ctxT, "w_qkv": W_qkv,
            "qpos": qpos_b, "kpos": np.ascontiguousarray(kpos),
            "bqT": bq, "bkT": bk, "bvb": bv,
            "onesd": np.ones((P, 8), bf16),
        })
    return in_maps


def assemble(results, B, N, D):
    out = np.zeros((B, N, D), np.float32)
    for c, res in enumerate(results):
        b, j = divmod(c, 2)
        order = _chunk_order(j, N // CH)
        o = np.asarray(res["out"], np.float32)
        out[b, order[0] * CH:(order[0] + 1) * CH] = o[:CH]
        out[b, order[3] * CH:(order[3] + 1) * CH] = o[CH:]
    return out


def run(inputs, trace=False, **spmd_kwargs):
    context = np.asarray(inputs["context"])
    B, N, D = context.shape
    nc = build(N, D)
    in_maps = make_in_maps(context, inputs["W_qkv"], inputs["b_qkv"], n_cores=8)
    res = run_bass_kernel_spmd(nc, in_maps, core_ids=list(range(8)), trace=trace, **spmd_kwargs)
    out = assemble(res.results, B, N, D)
    return out, res


def kernel(context, W_qkv, b_qkv):
    out, _ = run({"context": context, "W_qkv": W_qkv, "b_qkv": b_qkv})
    return out


# revision 8
# speedup vs baseline: 1.8918x; 1.8064x over previous
"""Causal self-attention (QKV projection + softmax(QK^T/sqrt(N)) @ V) on 8 TRN2
NeuronCores.

Sharding: core c = 2*b + j handles batch element b (of 4) and half the query
rows (two 512-row blocks from opposite ends of the causal triangle).

The kernel never materializes K or V. Both big projections are reassociated so
the per-core work is proportional to the core's OWN 1024 queries instead of
the full 2048-key sequence (which is duplicated across the core pair):

  scores = (ctx Wk + bk)(ctx Wq + bq)^T
         = ctx (Wk Wq^T) ctx^T + a_k + (q-terms that cancel in softmax)
    -> host folds Wkq^T = (Wq Wk^T)/sqrt(N) (weight-only), device computes
       U = Wkq^T ctx_q^T  [D x 512 per query block], then S^T = ctx^T.T-tiles
       @ U per key tile; a_k = ctx (Wk bq)/sqrt(N) is a host matvec shipped as
       a per-key-tile activation bias for the Exp.
  out   = P (ctx Wv + bv) = (P^T ctx) Wv + bv   (sum P = 1 after normalize)
    -> device computes op^T = ctx_rows^T-tiles @ P per d-tile, then
       out = op Wv / den + bv.

Per-core big (512-free) matmuls: 128 (U) + 192 (S) + 192 (P^T ctx) + 128 (Wv)
= 640, vs 1024 for the direct QKV formulation. All operands bf16 (f32 PSUM);
simulated end-to-end rel err ~4.5e-3 vs the 2e-2 gate.

Uniform-SPMD trick: each core processes the four 512-token chunks in a
per-core PERMUTED order (j=0: [0,1,2,3]; j=1: [1,0,3,2]) so the chunk at
schedule position 0 is always the core's low query block and position 3 its
high block; per-core causal masks built from shipped position vectors make the
uniform schedule correct. Everything is SBUF-resident (~18MB): the attention
phase does zero HBM traffic except output writes.
"""

import math
from contextlib import ExitStack

import numpy as np

import concourse.bass as bass
import concourse.mybir as mybir
import concourse.tile as tile
from concourse.bass_utils import run_bass_kernel_spmd

P = 128
CH = 512  # ctx chunk columns == query block rows == max f32 PSUM free dim


def _fix_matmul_waits(nc):
    """Walrus codegen has a small per-instruction sync-wait slot budget (one
    for a self-loading matmul's LDWEIGHTS half, similar for ACT etc). Move
    extra waits onto NoOps inserted just before the instruction on the same
    engine — per-engine program order (and thus semantics) is unchanged."""
    skip = (mybir.InstEventSemaphore, mybir.InstNoOp,
            mybir.InstUnconditionalBranch, mybir.InstCall)
    for func in nc.m.functions:
        for bb in func.blocks:
            il = bb.instructions
            new = []
            changed = False
            for inst in il:
                si = getattr(inst, "sync_info", None)
                if (si and si.on_wait and len(si.on_wait) > 1
                        and not isinstance(inst, skip)):
                    waits = list(si.on_wait)
                    for wi, w in enumerate(waits[:-1]):
                        nop = mybir.InstNoOp(
                            name=f"{inst.name}-wfix{wi}", engine=inst.engine,
                            sync_info=mybir.SyncInfo(on_wait=[w], on_update=[]),
                            text_hint="waitfix")
                        new.append(nop)
                    inst.sync_info = mybir.SyncInfo(
                        on_wait=[waits[-1]], on_update=list(si.on_update or []))
                    changed = True
                new.append(inst)
            if changed:
                bb.instructions = new


def build(N=2048, D=1024, fix_waits=True, **bass_kwargs):
    NT = N // P          # schedule k-tiles
    DN = D // P          # 128-blocks of the model dim
    NCH = N // CH        # ctx chunks == query blocks per batch
    QBLK = CH
    QT = QBLK // P
    QTOT = 2 * QBLK
    BF = mybir.dt.bfloat16
    F32 = mybir.dt.float32
    AF = mybir.ActivationFunctionType
    OP = mybir.AluOpType

    nc = bass.Bass(**bass_kwargs)

    ctxT = nc.declare_dram_parameter("ctxT", [D, N], BF, isOutput=False)
    ctxR = nc.declare_dram_parameter("ctxR", [N, D], BF, isOutput=False)
    wkqT = nc.declare_dram_parameter("wkqT", [D, D], BF, isOutput=False)
    wvd = nc.declare_dram_parameter("wvd", [D, D], BF, isOutput=False)
    qpos = nc.declare_dram_parameter("qpos", [P, QTOT], F32, isOutput=False)
    kpos = nc.declare_dram_parameter("kpos", [P, NT], F32, isOutput=False)
    abk = nc.declare_dram_parameter("abk", [P, NT], F32, isOutput=False)
    bvb = nc.declare_dram_parameter("bvb", [P, D], F32, isOutput=False)
    onesd = nc.declare_dram_parameter("onesd", [P, 8], BF, isOutput=False)
    out_ext = nc.declare_dram_parameter("out", [QTOT, D], BF, isOutput=True)

    with ExitStack() as ctx:
        tc = ctx.enter_context(tile.TileContext(nc))
        const = ctx.enter_context(tc.tile_pool(name="const", bufs=1))
        persist = ctx.enter_context(tc.tile_pool(name="persist", bufs=1))
        mpool = ctx.enter_context(tc.tile_pool(name="mp", bufs=3))
        rpool = ctx.enter_context(tc.tile_pool(name="rp", bufs=1))
        opool = ctx.enter_context(tc.tile_pool(name="op", bufs=3))
        pp = ctx.enter_context(tc.tile_pool(name="pp", bufs=4, space="PSUM"))
        dpp = ctx.enter_context(tc.tile_pool(name="dpp", bufs=1, space="PSUM"))

        # SBUF stores (bf16):
        #   wkq_sb: lhsT tile (t_out, c_contract) at col (t*DN+c)*P      (2MB)
        #   cts[pos]: ctx^T chunk, col d*CH + token                  (4x 1MB)
        #   ctr_sb: ctx rows, col kt*D + d                               (4MB)
        #   wv_sb: d-major, col d*D + e                                  (2MB)
        #   u_store: col t*QTOT + qb*QBLK + q                            (2MB)
        #   e_all: col kt*QBLK + q                                       (2MB)
        #   op_sb: col d*QBLK + q (per-qb scratch)                       (1MB)
        wkq_sb = persist.tile([P, D * DN], BF, name="wkq")
        wv_sb = persist.tile([P, D * DN], BF, name="wv")
        cts = [persist.tile([P, DN * CH], BF, name=f"ct{c}") for c in range(NCH)]
        ctr_sb = persist.tile([P, NT * D], BF, name="ctr")
        u_store = persist.tile([P, DN * QTOT], BF, name="u_store")
        e_all = persist.tile([P, NT * QBLK], BF, name="e_all")
        op_sb = persist.tile([P, DN * QBLK], BF, name="op_sb")

        qpos_sb = const.tile([P, QTOT], F32)
        kpos_sb = const.tile([P, NT], F32)
        ab_sb = const.tile([P, NT], F32)
        bv_sb = const.tile([P, D], F32)
        ones_sb = const.tile([P, 8], BF)

        # ---- DMA stream, in exact consumption order (one sync ring) ----
        def ld_colblk(dst, src, t):  # e/t-column block t of a [D, D] matrix
            nc.sync.dma_start(
                out=dst[:, t * D:(t + 1) * D].rearrange("p (c x) -> p c x", x=P),
                in_=src[:, t * P:(t + 1) * P].rearrange("(c p) x -> p c x", p=P))

        def ld_ctx(pos):
            nc.sync.dma_start(
                out=cts[pos].rearrange("p (d c) -> p d c", c=CH),
                in_=ctxT[:, pos * CH:(pos + 1) * CH].rearrange("(d p) c -> p d c", p=P))

        ld_colblk(wkq_sb, wkqT, 0)
        ld_ctx(0)
        for t in range(1, DN):
            ld_colblk(wkq_sb, wkqT, t)
        ld_ctx(3)                    # U(qb1) source
        nc.sync.dma_start(out=ab_sb, in_=abk[:, :])
        ld_ctx(1)
        nc.sync.dma_start(out=qpos_sb, in_=qpos[:, :])
        nc.sync.dma_start(out=kpos_sb, in_=kpos[:, :])
        nc.sync.dma_start(out=ones_sb, in_=onesd[:, :])
        ld_ctx(2)
        nc.sync.dma_start(            # ctx rows, schedule-k-tile-permuted
            out=ctr_sb.rearrange("p (t d) -> p t d", d=D),
            in_=ctxR.rearrange("(t p) d -> p t d", p=P))
        for d in range(DN):           # wv d-major
            nc.sync.dma_start(out=wv_sb[:, d * D:(d + 1) * D],
                              in_=wvd[d * P:(d + 1) * P, :])
        nc.sync.dma_start(out=bv_sb, in_=bvb[:, :])

        # ---- U = Wkq^T @ ctx_q^T for both query blocks ----
        for qb, pos in ((0, 0), (1, NCH - 1)):
            for t in range(DN):
                psu = pp.tile([P, CH], F32, tag="big", name="psu")
                for c in range(DN):
                    nc.tensor.matmul(
                        psu, lhsT=wkq_sb[:, (t * DN + c) * P:(t * DN + c + 1) * P],
                        rhs=cts[pos][:, c * CH:(c + 1) * CH],
                        start=(c == 0), stop=(c == DN - 1))
                nc.scalar.activation(
                    u_store[:, t * QTOT + qb * QBLK:t * QTOT + (qb + 1) * QBLK],
                    psu, AF.Copy)

        # ---- attention ----
        for qb in range(2):
            KT = NT // 2 if qb == 0 else NT
            # scores + exp + mask
            for k in range(KT):
                pos, loc = divmod(k, CH // P)
                pss = pp.tile([P, QBLK], F32, tag="big", name="pss")
                for d in range(DN):
                    nc.tensor.matmul(
                        pss, lhsT=cts[pos][:, d * CH + loc * P:d * CH + (loc + 1) * P],
                        rhs=u_store[:, d * QTOT + qb * QBLK:d * QTOT + (qb + 1) * QBLK],
                        start=(d == 0), stop=(d == DN - 1))
                esl = e_all[:, k * QBLK:(k + 1) * QBLK]
                nc.scalar.activation(esl, pss, AF.Exp, bias=ab_sb[:, k:k + 1])
                if qb == 0 or k >= NT // 2:
                    m = mpool.tile([P, QBLK], BF, tag="m", name="m")
                    nc.vector.tensor_scalar(m, qpos_sb[:, qb * QBLK:(qb + 1) * QBLK],
                                            kpos_sb[:, k:k + 1], None, OP.is_ge)
                    nc.vector.tensor_tensor(esl, esl, m, OP.mult)
            # denominators (psd[qt] accumulates over k) + reciprocals
            psd = [dpp.tile([P, 8], F32, tag=f"den{qt}", name="psd") for qt in range(QT)]
            for qt in range(QT):
                for k in range(KT):
                    nc.tensor.matmul(psd[qt],
                                     lhsT=e_all[:, k * QBLK + qt * P:k * QBLK + (qt + 1) * P],
                                     rhs=ones_sb, start=(k == 0), stop=(k == KT - 1))
            recs = []
            for qt in range(QT):
                rec = rpool.tile([P, 1], F32, tag=f"rec{qt}", name="rec")
                nc.vector.reciprocal(rec, psd[qt][:, 0:1])
                recs.append(rec)
            # op^T = ctx_rows^T-tiles @ P  (d-tile at a time, 1 PSUM bank each)
            for d in range(DN):
                ppv = pp.tile([P, QBLK], F32, tag="big", name="ppv")
                for k in range(KT):
                    nc.tensor.matmul(
                        ppv, lhsT=ctr_sb[:, k * D + d * P:k * D + (d + 1) * P],
                        rhs=e_all[:, k * QBLK:(k + 1) * QBLK],
                        start=(k == 0), stop=(k == KT - 1))
                nc.scalar.activation(op_sb[:, d * QBLK:(d + 1) * QBLK], ppv, AF.Copy)
            # out = op Wv / den + bv
            for qt in range(QT):
                for ei in range(D // CH):
                    psf = pp.tile([P, CH], F32, tag="big", name="psf")
                    for d in range(DN):
                        nc.tensor.matmul(
                            psf, lhsT=op_sb[:, d * QBLK + qt * P:d * QBLK + (qt + 1) * P],
                            rhs=wv_sb[:, d * D + ei * CH:d * D + (ei + 1) * CH],
                            start=(d == 0), stop=(d == DN - 1))
                    ot = opool.tile([P, CH], BF, tag="o", name="ot")
                    nc.vector.tensor_scalar_mul(ot, psf, recs[qt])
                    nc.vector.tensor_tensor(ot, ot, bv_sb[:, ei * CH:(ei + 1) * CH], OP.add)
                    nc.scalar.dma_start(
                        out=out_ext[qb * QBLK + qt * P:qb * QBLK + (qt + 1) * P,
                                    ei * CH:(ei + 1) * CH],
                        in_=ot)
    if fix_waits:
        _fix_matmul_waits(nc)
    return nc


def _chunk_order(j):
    # schedule position 0 = low query block, position 3 = high query block.
    return [0, 1, 2, 3] if j == 0 else [1, 0, 3, 2]


def make_in_maps(context, W_qkv, b_qkv, n_cores=8):
    import ml_dtypes
    bf16 = ml_dtypes.bfloat16
    context = np.ascontiguousarray(np.asarray(context, np.float32))
    W_qkv = np.asarray(W_qkv, np.float32)
    b_qkv = np.asarray(b_qkv, np.float32)
    B, N, D = context.shape
    NT = N // P
    QTOT = 2 * CH
    SCALE = 1.0 / math.sqrt(N)
    Wq, Wk, Wv = W_qkv[:, :D], W_qkv[:, D:2 * D], W_qkv[:, 2 * D:]
    bq, bk, bv = b_qkv[:D], b_qkv[D:2 * D], b_qkv[2 * D:]
    wkqT = np.ascontiguousarray(((Wq @ Wk.T) * SCALE).astype(bf16))
    wvd = np.ascontiguousarray(Wv.astype(bf16))
    bvb = np.ascontiguousarray(np.broadcast_to(bv, (P, D)).astype(np.float32))
    wkbq = (Wk @ bq) * SCALE  # [D]; a_k = ctx_k . wkbq (k-dependent exp bias)
    in_maps = []
    for c in range(n_cores):
        b, j = divmod(c, 2)
        order = _chunk_order(j)
        ctx_b = context[b]
        ctx_bT = ctx_b.T.astype(bf16)
        ctxT = np.ascontiguousarray(np.concatenate(
            [ctx_bT[:, o * CH:(o + 1) * CH] for o in order], axis=1))
        ctxR = np.ascontiguousarray(np.concatenate(
            [ctx_b[o * CH:(o + 1) * CH] for o in order], axis=0).astype(bf16))
        qpos_row = np.concatenate([
            np.arange(order[0] * CH, (order[0] + 1) * CH),
            np.arange(order[3] * CH, (order[3] + 1) * CH)]).astype(np.float32)
        qpos_b = np.ascontiguousarray(np.broadcast_to(qpos_row, (P, QTOT)))
        kpos = np.empty((P, NT), np.float32)
        abk = np.empty((P, NT), np.float32)
        a_full = ctx_b @ wkbq  # [N]
        for t in range(NT):
            keys = order[t // 4] * CH + (t % 4) * P + np.arange(P)
            kpos[:, t] = keys
            abk[:, t] = a_full[keys]
        in_maps.append({
            "ctxT": ctxT, "ctxR": ctxR, "wkqT": wkqT, "wvd": wvd,
            "qpos": qpos_b, "kpos": np.ascontiguousarray(kpos),
            "abk": np.ascontiguousarray(abk), "bvb": bvb,
            "onesd": np.ones((P, 8), bf16),
        })
    return in_maps


def assemble(results, B, N, D):
    out = np.zeros((B, N, D), np.float32)
    for c, res in enumerate(results):
        b, j = divmod(c, 2)
        order = _chunk_order(j)
        o = np.asarray(res["out"], np.float32)
        out[b, order[0] * CH:(order[0] + 1) * CH] = o[:CH]
        out[b, order[3] * CH:(order[3] + 1) * CH] = o[CH:]
    return out


def run(inputs, trace=False, **spmd_kwargs):
    context = np.asarray(inputs["context"])
    B, N, D = context.shape
    nc = build(N, D)
    in_maps = make_in_maps(context, inputs["W_qkv"], inputs["b_qkv"], n_cores=8)
    res = run_bass_kernel_spmd(nc, in_maps, core_ids=list(range(8)), trace=trace, **spmd_kwargs)
    out = assemble(res.results, B, N, D)
    return out, res


def kernel(context, W_qkv, b_qkv):
    out, _ = run({"context": context, "W_qkv": W_qkv, "b_qkv": b_qkv})
    return out
